# revision 1
# baseline (speedup 1.0000x reference)
# Trainium2 Bass kernel for nn_CapLayer (CapsNet grouped 1x1 conv + dynamic routing).
#
# Key algebraic restructuring: the huge intermediate pred[b, i=(g,s), (j,d)]
# (188MB for the full batch) is NEVER materialized. Routing is computed in a
# factored form:
#   pred[b,(g,s),(j,d)] = sum_c Wa[g,j,d,c] * xga[b,g,c,s]     (c augmented with
#                                                               a ones channel to
#                                                               absorb the bias)
#   t[b,j,g,c]  = sum_s c[b,j,(g,s)] * xga[b,g,c,s]
#   s[b,j,d]    = sum_{g,c} t[b,j,g,c] * Wa[g,j,d,c]
#   u[b,j,g,c]  = sum_d v[b,j,d] * Wa[g,j,d,c]
#   db[b,j,g,s] = sum_c u[b,j,g,c] * xga[b,g,c,s]
# Iteration 1 collapses (softmax of zeros is uniform): t1 = xsum / J.
#
# Sharding: pure data parallel, 32 samples per core across 8 cores.
# On-chip layout: partition p = (b4, g) with 4 samples x 32 groups = 128
# partitions; 8 chunks cover the 32 local samples. The g-contraction for
# s[b,(j,d)] is done on the TensorEngine with a block-diagonal ones matrix,
# which also replicates s across the g-partitions for free (so v and u stay
# in the same partition layout).
#
# Engine split: broadcast-products run in bf16 (DVE 2x mode / GPSIMD),
# segmented reductions and small elementwise stay on DVE in fp32 accuracy,
# exp/sqrt/copies ride the Scalar engine, the g-sum is a TensorE matmul.

import sys

import numpy as np

# concourse (Bass/Tile) ships with the container; make sure it's importable
# when the grader runs kernel.py from a bare directory.
for _p in ("/opt/trn_rl_repo", "/root/.axon_site/_ro/trn_rl_repo"):
    if _p not in sys.path:
        sys.path.insert(0, _p)

NS, J, D, C_IN, H, WID, RN = 32, 10, 16, 8, 6, 6, 3
S = H * WID            # 36 spatial positions
CA = C_IN + 1          # 9 channels including the ones channel
CP = 10                # padded channel stride (4B alignment for bf16 rows)
NCORES = 8
BLOC = 32              # samples per core
B4 = 4                 # samples per chunk
NCH = BLOC // B4       # 8 chunks

_CACHE = {}


def _build_program(split_waits=True, dve_chunks=8, dma_eng="sync"):
    from contextlib import ExitStack

    import concourse.bass as bass
    import concourse.tile as tile
    from concourse import mybir

    f32 = mybir.dt.float32
    bf16 = mybir.dt.float16
    Alu = mybir.AluOpType
    Act = mybir.ActivationFunctionType
    AxX = mybir.AxisListType.X

    nc = bass.Bass("TRN2", target_bir_lowering=True, debug=False,
                   num_devices=NCORES)

    xcs_d = nc.dram_tensor("xcs", [NCH, 128, CA * S], bf16,
                           kind="ExternalInput").ap()      # free = (c, s)
    xsc_d = nc.dram_tensor("xsc", [NCH, 128, S * CP], bf16,
                           kind="ExternalInput").ap()      # free = (s, c10)
    wc_d = nc.dram_tensor("wc", [128, J * D * CP], bf16,
                          kind="ExternalInput").ap()       # free = (j, d, c10)
    wu_d = nc.dram_tensor("wu", [128, J * CA * D], bf16,
                          kind="ExternalInput").ap()       # free = (j, c, d)
    ones_d = nc.dram_tensor("onesb", [128, 128], bf16,
                            kind="ExternalInput").ap()     # blockdiag over b4
    v_d = nc.dram_tensor("v", [BLOC, J * D], f32,
                         kind="ExternalOutput").ap()

    dmae = {"gpsimd": nc.gpsimd, "sync": nc.sync}[dma_eng]
    with tile.TileContext(nc) as tc, ExitStack() as ctx:
        consts = ctx.enter_context(tc.tile_pool(name="consts", bufs=1))
        xpool = ctx.enter_context(tc.tile_pool(name="xpool", bufs=1))
        lpool = ctx.enter_context(tc.tile_pool(name="lpool", bufs=1))
        spool = ctx.enter_context(tc.tile_pool(name="scratch", bufs=2))
        small = ctx.enter_context(tc.tile_pool(name="small", bufs=3))
        vpool = ctx.enter_context(tc.tile_pool(name="vpool", bufs=2))
        psum = ctx.enter_context(tc.tile_pool(name="psum", bufs=2,
                                              space="PSUM"))

        wc_t = consts.tile([128, J * D * CP], bf16, tag="wc")
        dmae.dma_start(wc_t[:, :], wc_d[:, :])
        wu_t = consts.tile([128, J * CA * D], bf16, tag="wu")
        dmae.dma_start(wu_t[:, :], wu_d[:, :])
        ones_t = consts.tile([128, 128], bf16, tag="onesb")
        dmae.dma_start(ones_t[:, :], ones_d[:, :])

        # Persistent per-chunk tiles.
        Xcs = []   # xga [p, (c, s)] bf16
        Xsc = []   # xga [p, (s, c)] bf16
        L = []     # routing logits b, layout [p, (j, s)] fp32
        for ch in range(NCH):
            xt = xpool.tile([128, CA * S], bf16, tag=f"Xcs{ch}",
                            name=f"Xcs{ch}")
            dmae.dma_start(xt[:, :], xcs_d[ch, :, :])
            Xcs.append(xt)
            xt2 = xpool.tile([128, S * CP], bf16, tag=f"Xsc{ch}",
                             name=f"Xsc{ch}")
            dmae.dma_start(xt2[:, :], xsc_d[ch, :, :])
            Xsc.append(xt2)
            L.append(lpool.tile([128, J * S], f32, tag=f"L{ch}",
                                name=f"L{ch}"))

        def prod_engine(ch):
            # Split the broadcast-product work between DVE and GPSIMD by
            # chunk so both engines stay busy.
            return nc.vector if (ch % 8) < dve_chunks else nc.gpsimd

        def c_step(ch, t_in0_bcast):
            """t x Wa summed over (g, c) -> replicated s [p, (j,d)].

            t_in0_bcast: AP broadcast to [p, J, D, CA] (bf16).
            Returns an SBUF tile [128, J*D] fp32 with s replicated over g
            within each b4 partition block.
            """
            eng = prod_engine(ch)
            pc = spool.tile([128, J * D * CP], bf16, tag="prodC")
            pc4 = (pc[:, :].rearrange("p (j d c) -> p j d c", j=J, d=D)
                   [:, :, :, 0:CA])
            wc4 = (wc_t[:, :].rearrange("p (j d c) -> p j d c", j=J, d=D)
                   [:, :, :, 0:CA])
            eng.tensor_tensor(pc4, t_in0_bcast, wc4, Alu.mult)
            # PE contracts g (partitions, via blockdiag ones) AND c (PSUM
            # accumulation over the 9 channel slices) in one group -- no
            # DVE reduction needed at all.
            pcz = pc[:, :].rearrange("p (a c) -> p a c", c=CP)
            ps = psum.tile([128, J * D], f32, tag="psum_s")
            for c in range(CA):
                nc.tensor.matmul(ps[:, :], ones_t[:, :], pcz[:, :, c],
                                 start=(c == 0), stop=(c == CA - 1))
            s_sb = small.tile([128, J * D], f32, tag="s_sb")
            nc.scalar.copy(s_sb[:, :], ps[:, :])
            return ps, s_sb

        def squash(ch, s_ps, s_sb, want_bf16):
            """v = s * |s| / (1 + |s|^2), norm over d."""
            s2 = small.tile([128, J * D], f32, tag="s2")
            nc.scalar.activation(s2[:, :], s_ps[:, :], Act.Square)
            n2 = small.tile([128, J], f32, tag="n2")
            nc.vector.tensor_reduce(
                n2[:, :], s2[:, :].rearrange("p (j d) -> p j d", j=J), AxX,
                Alu.add)
            n2p1 = small.tile([128, J], f32, tag="n2p1")
            nc.scalar.add(n2p1[:, :], n2[:, :], 1.0)
            r = small.tile([128, J], f32, tag="rcp")
            nc.vector.reciprocal(r[:, :], n2p1[:, :])
            nr = small.tile([128, J], f32, tag="nrm")
            nc.scalar.activation(nr[:, :], n2[:, :], Act.Sqrt)
            f = small.tile([128, J], f32, tag="fac")
            nc.vector.tensor_tensor(f[:, :], nr[:, :], r[:, :], Alu.mult)
            fb = f[:, :].unsqueeze(2).broadcast_to([128, J, D])
            if want_bf16:
                vt = vpool.tile([128, J * D], bf16, tag="vtb")
            else:
                vt = vpool.tile([128, J * D], f32, tag="vtf")
            nc.vector.tensor_tensor(
                vt[:, :].rearrange("p (j d) -> p j d", j=J),
                s_sb[:, :].rearrange("p (j d) -> p j d", j=J), fb, Alu.mult)
            return vt

        def u_step(ch, vt):
            """u[p,(j,c)] = sum_d v[p,(j,d)] * Wa[p,(j,c,d)]. Out bf16
            padded to stride CP."""
            eng = prod_engine(ch)
            pu = spool.tile([128, J * CA * D], bf16, tag="produ")
            pu4 = pu[:, :].rearrange("p (j c d) -> p j c d", j=J, c=CA)
            wu4 = wu_t[:, :].rearrange("p (j c d) -> p j c d", j=J, c=CA)
            vb = (vt[:, :].rearrange("p (j d) -> p j d", j=J)
                  .unsqueeze(2).broadcast_to([128, J, CA, D]))
            eng.tensor_tensor(pu4, vb, wu4, Alu.mult)
            puz = pu[:, :].rearrange("p (a d) -> p a d", d=D)
            uA = spool.tile([128, 90 * 8], bf16, tag="treeUA")
            uA3 = uA[:, :].rearrange("p (a c) -> p a c", c=8)
            nc.vector.tensor_tensor(uA3, puz[:, :, 0:8], puz[:, :, 8:16],
                                    Alu.add)
            uB = spool.tile([128, 90 * 4], bf16, tag="treeUB")
            uB3 = uB[:, :].rearrange("p (a c) -> p a c", c=4)
            nc.vector.tensor_tensor(uB3, uA3[:, :, 0:4], uA3[:, :, 4:8],
                                    Alu.add)
            uC = spool.tile([128, 90 * 2], bf16, tag="treeUC")
            uC3 = uC[:, :].rearrange("p (a c) -> p a c", c=2)
            nc.vector.tensor_tensor(uC3, uB3[:, :, 0:2], uB3[:, :, 2:4],
                                    Alu.add)
            u = small.tile([128, J * CP], bf16, tag="u")
            u3 = u[:, :].rearrange("p (j c) -> p j c", j=J)[:, :, 0:CA]
            nc.vector.tensor_tensor(u3, uC3[:, :, 0], uC3[:, :, 1],
                                    Alu.add)
            return u

        def e_heavy(ch, u, out_js):
            """db[p,(j,s)] = sum_c u[p,(j,c)] * x[p,(s,c)] -> out_js fp32."""
            eng = prod_engine(ch)
            pe = spool.tile([128, J * S * CP], bf16, tag="prodE")
            pe4 = (pe[:, :].rearrange("p (j s c) -> p j s c", j=J, s=S)
                   [:, :, :, 0:CA])
            ub = (u[:, :].rearrange("p (j c) -> p j c", j=J)[:, :, 0:CA]
                  .unsqueeze(2).broadcast_to([128, J, S, CA]))
            xb = (Xsc[ch][:, :].rearrange("p (s c) -> p s c", s=S)
                  [:, :, 0:CA].unsqueeze(1)
                  .broadcast_to([128, J, S, CA]))
            eng.tensor_tensor(pe4, ub, xb, Alu.mult)
            # pe layout (j, s, c10): (j,s) merges; tree-sum over c
            pez = pe[:, :].rearrange("p (a c) -> p a c", c=CP)
            eA = spool.tile([128, 360 * 4], bf16, tag="treeEA")
            eA3 = eA[:, :].rearrange("p (a c) -> p a c", c=4)
            nc.vector.tensor_tensor(eA3, pez[:, :, 0:4], pez[:, :, 4:8],
                                    Alu.add)
            eB = spool.tile([128, 360 * 2], bf16, tag="treeEB")
            eB3 = eB[:, :].rearrange("p (a c) -> p a c", c=2)
            nc.vector.tensor_tensor(eB3, eA3[:, :, 0:2], eA3[:, :, 2:4],
                                    Alu.add)
            nc.vector.tensor_tensor(out_js, eB3[:, :, 0], eB3[:, :, 1],
                                    Alu.add)
            nc.vector.scalar_tensor_tensor(out_js, pez[:, :, 8], 1.0,
                                           out_js, Alu.mult, Alu.add)

        def softmax(ch):
            """c[p,(j,s)] = softmax_j(L). Returns bf16 C tile."""
            et = spool.tile([128, J * S], f32, tag="expt")
            nc.scalar.activation(et[:, :], L[ch][:, :], Act.Exp)
            z = small.tile([128, S], f32, tag="z")
            # reduce over j: view [p, s(outer, stride 1), j(inner, stride S)]
            ejs = (et[:, :].rearrange("p (j s) -> p j s", j=J)
                   .transpose([0, 2, 1]))
            nc.vector.tensor_reduce(z[:, :], ejs, AxX, Alu.add)
            zr = small.tile([128, S], f32, tag="zr")
            nc.vector.reciprocal(zr[:, :], z[:, :])
            ct = spool.tile([128, J * S], bf16, tag="ct")
            zb = zr[:, :].unsqueeze(1).broadcast_to([128, J, S])
            nc.vector.tensor_tensor(
                ct[:, :].rearrange("p (j s) -> p j s", j=J),
                et[:, :].rearrange("p (j s) -> p j s", j=J), zb, Alu.mult)
            return ct

        def b_heavy(ch, ct):
            """t[p,(j,c)] = sum_s c[p,(j,s)] * x[p,(c,s)]. Out bf16 padded
            to stride CP."""
            eng = prod_engine(ch)
            pb = spool.tile([128, J * CA * S], bf16, tag="prodB")
            pb4 = pb[:, :].rearrange("p (j c s) -> p j c s", j=J, c=CA)
            cb = (ct[:, :].rearrange("p (j s) -> p j s", j=J)
                  .unsqueeze(2).broadcast_to([128, J, CA, S]))
            xb = (Xcs[ch][:, :].rearrange("p (c s) -> p c s", c=CA)
                  .unsqueeze(1).broadcast_to([128, J, CA, S]))
            eng.tensor_tensor(pb4, cb, xb, Alu.mult)
            pbz = pb[:, :].rearrange("p (a s) -> p a s", s=S)
            bA = spool.tile([128, 90 * 16], bf16, tag="treeBA")
            bA3 = bA[:, :].rearrange("p (a c) -> p a c", c=16)
            nc.vector.tensor_tensor(bA3, pbz[:, :, 0:16], pbz[:, :, 16:32],
                                    Alu.add)
            bB = spool.tile([128, 90 * 8], bf16, tag="treeBB")
            bB3 = bB[:, :].rearrange("p (a c) -> p a c", c=8)
            nc.vector.tensor_tensor(bB3, bA3[:, :, 0:8], bA3[:, :, 8:16],
                                    Alu.add)
            bC = spool.tile([128, 90 * 4], bf16, tag="treeBC")
            bC3 = bC[:, :].rearrange("p (a c) -> p a c", c=4)
            nc.vector.tensor_tensor(bC3, bB3[:, :, 0:4], bB3[:, :, 4:8],
                                    Alu.add)
            # tail s=32..35 pairs
            bT = spool.tile([128, 90 * 2], bf16, tag="treeBT")
            bT3 = bT[:, :].rearrange("p (a c) -> p a c", c=2)
            nc.vector.tensor_tensor(bT3, pbz[:, :, 32:34], pbz[:, :, 34:36],
                                    Alu.add)
            bD = spool.tile([128, 90 * 2], bf16, tag="treeBD")
            bD3 = bD[:, :].rearrange("p (a c) -> p a c", c=2)
            nc.vector.tensor_tensor(bD3, bC3[:, :, 0:2], bC3[:, :, 2:4],
                                    Alu.add)
            bE = spool.tile([128, 90 * 2], f32, tag="treeBE")
            bE3 = bE[:, :].rearrange("p (a c) -> p a c", c=2)
            nc.vector.tensor_tensor(bE3, bD3[:, :, :], bT3[:, :, :],
                                    Alu.add)
            t = small.tile([128, J * CP], bf16, tag="tt")
            t3 = t[:, :].rearrange("p (j c) -> p j c", j=J)[:, :, 0:CA]
            nc.vector.tensor_tensor(t3, bE3[:, :, 0], bE3[:, :, 1],
                                    Alu.add)
            return t

        def t_bcast(t):
            """[p, (j, c-padded)] bf16 -> broadcast AP [p, J, D, CA]."""
            return (t[:, :].rearrange("p (j c) -> p j c", j=J)[:, :, 0:CA]
                    .unsqueeze(2).broadcast_to([128, J, D, CA]))

        for ch in range(NCH):
            # ---- iteration 1 (uniform c = 1/J) ----
            xsum = small.tile([128, CA], bf16, tag="xsum")
            with nc.allow_low_precision("bf16 routing intermediates"):
                nc.vector.tensor_reduce(
                    xsum[:, :],
                    Xcs[ch][:, :].rearrange("p (c s) -> p c s", c=CA), AxX,
                    Alu.add)
            xs1 = small.tile([128, CA], bf16, tag="xsum1")
            nc.scalar.mul(xs1[:, :], xsum[:, :], 1.0 / J)
            xs_b = (xs1[:, :].unsqueeze(1).unsqueeze(1)
                    .broadcast_to([128, J, D, CA]))
            s_ps, s_sb = c_step(ch, xs_b)
            vt = squash(ch, s_ps, s_sb, want_bf16=True)
            u = u_step(ch, vt)
            e_heavy(ch, u, L[ch][:, :])  # L = db1  (b was zero)

            # ---- iteration 2 ----
            ct = softmax(ch)
            t = b_heavy(ch, ct)
            s_ps, s_sb = c_step(ch, t_bcast(t))
            vt = squash(ch, s_ps, s_sb, want_bf16=True)
            u = u_step(ch, vt)
            db = spool.tile([128, J * S], f32, tag="db2")
            e_heavy(ch, u, db[:, :])
            nc.vector.tensor_tensor(L[ch][:, :], L[ch][:, :], db[:, :],
                                    Alu.add)

            # ---- iteration 3 (only v needed) ----
            ct = softmax(ch)
            t = b_heavy(ch, ct)
            s_ps, s_sb = c_step(ch, t_bcast(t))
            vt = squash(ch, s_ps, s_sb, want_bf16=False)
            dmae.dma_start(v_d[ch * B4:(ch + 1) * B4, :],
                                vt[0:128:NS, :])

    if split_waits:
        _split_multi_waits(nc)
    return nc


def _split_multi_waits(nc):
    """Walrus's cayman codegen allows exactly ONE sync wait per TPB
    instruction (NEURON_ISA_TPB_EVENTS has a single wait slot). Tile's
    scheduler attaches several waits to dependency-merge instructions,
    which the native bass encoder handles but the neuronx-cc path rejects
    ("Too many sync wait commands"). Split the extras onto engine-local
    NoOp instructions inserted immediately before the owner so the wait
    semantics are unchanged.
    """
    from concourse import mybir

    for bbname, bbwrap in nc.bb_map.items():
        bb = bbwrap.bb
        insts = bb.instructions
        i = 0
        while i < len(insts):
            ins = insts[i]
            si = getattr(ins, "sync_info", None)
            if si is None or len(si.on_wait or []) <= 1:
                i += 1
                continue
            waits = list(si.on_wait)
            engine = ins.engine
            for w in waits[:-1]:
                nop = mybir.InstNoOp(
                    name=nc.get_next_instruction_name(),
                    engine=engine,
                    bass_nofuse=True,
                    sync_info=mybir.SyncInfo(on_wait=[w], on_update=[]),
                )
                insts.insert(i, nop)
                i += 1
            ins.sync_info = mybir.SyncInfo(on_wait=[waits[-1]],
                                           on_update=si.on_update)
            i += 1


def _get_program(split_waits=True, dve_chunks=8, dma_eng="sync"):
    key = ("nc", split_waits, dve_chunks, dma_eng)
    if key not in _CACHE:
        _CACHE[key] = _build_program(split_waits, dve_chunks, dma_eng)
    return _CACHE[key]


def _host_prep(x, W, bias):
    """Build per-core input maps."""
    bf = np.float16
    x = np.ascontiguousarray(x, dtype=np.float32)
    W = np.ascontiguousarray(W, dtype=np.float32)
    bias = np.ascontiguousarray(bias, dtype=np.float32)
    bs = x.shape[0]

    xga = x.reshape(bs, NS, C_IN, S)
    xa = np.concatenate(
        [xga, np.ones((bs, NS, 1, S), dtype=np.float32)], axis=2)
    # [core, ch, b4, g, c, s]
    x6 = xa.reshape(NCORES, NCH, B4, NS, CA, S)
    xcs = np.ascontiguousarray(x6).reshape(
        NCORES, NCH, 128, CA * S).astype(bf)
    x6sc = x6.transpose(0, 1, 2, 3, 5, 4)      # [.., s, c]
    x6sp = np.concatenate(
        [x6sc, np.zeros(x6sc.shape[:-1] + (CP - CA,), np.float32)], axis=-1)
    xsc = np.ascontiguousarray(x6sp).reshape(
        NCORES, NCH, 128, S * CP).astype(bf)

    Wa = np.concatenate(
        [W.reshape(NS, J, D, C_IN),
         bias.reshape(NS, J, D, 1)], axis=3)            # [g, j, d, c]
    Wap = np.concatenate(
        [Wa, np.zeros(Wa.shape[:-1] + (CP - CA,), np.float32)], axis=-1)
    wc = np.tile(Wap.reshape(NS, J * D * CP), (B4, 1)).astype(bf)
    wu = np.tile(
        Wa.transpose(0, 1, 3, 2).reshape(NS, J * CA * D),
        (B4, 1)).astype(bf)                             # [128, (j,c,d)]
    onesb = np.kron(np.eye(B4, dtype=np.float32),
                    np.ones((NS, NS), dtype=np.float32)).astype(bf)

    in_maps = [
        {"xcs": np.ascontiguousarray(xcs[k]),
         "xsc": np.ascontiguousarray(xsc[k]),
         "wc": wc, "wu": wu, "onesb": onesb}
        for k in range(NCORES)
    ]
    return in_maps


def kernel(x, W, bias, b0):
    from concourse.bass_utils import run_bass_kernel_spmd

    nc = _get_program()
    in_maps = _host_prep(x, W, bias)
    res = run_bass_kernel_spmd(nc, in_maps, list(range(NCORES)))
    out = np.concatenate([res.results[k]["v"] for k in range(NCORES)],
                         axis=0)
    return np.ascontiguousarray(out.reshape(NCORES * BLOC, J, D))



# revision 5
# speedup vs baseline: 4.5127x; 4.5127x over previous
# Trainium2 Bass kernel for nn_CapLayer (CapsNet grouped 1x1 conv + dynamic
# routing), PE-centric formulation.
#
# Math: pred[b,i=(g,s),(j,d)] = sum_c xa[b,g,c,s] * Wa[g,j,d,c]  (xa has a
# ones channel absorbing the bias). Routing never materializes pred or the
# logits b as state:
#   logits B[b,j,i] = <Vacc[b,j,:], pred[b,i,j,:]>   with Vacc = sum of v's
#                   = sum_c u_acc[b,j,g,c] * xa[b,g,c,s]
#   u_acc[b,j,g,c]  = sum_d Vacc[b,j,d] * Wa[g,j,d,c]
#   t[b,j,g,c]      = sum_s C[b,j,(g,s)] * xa[b,g,c,s]    (C = softmax_j B)
#   s[b,j,d]        = sum_{g,c} t[b,j,g,c] * Wa[g,j,d,c]
# Every contraction above runs on the TensorEngine as small matmuls whose
# stationary operand is either a host-staged per-(sample, i-chunk) slice of x
# (block-diagonal over capsule groups g) or a static arrangement of Wa. The
# i-space (1152 positions) is split into 9 chunks of 128 partitions; the
# (g,c)-space is laid out as 9 chunk-local 64-row slots (2 per 128-partition
# tile, PE quadrant-aligned), with boundary-g rows duplicated across chunks
# so cross-chunk partial sums resolve inside the next matmul's contraction.
#
# Per core (pure data parallel over batch, 32 samples/core), per routing
# iteration: 288 e-matmuls produce the logits straight into PSUM, softmax
# runs on Act (exp) + DVE (sum/recip/mul) in a [i-chunk, (j,b)] layout with
# b innermost (keeps DVE 2x mode), 288 b-matmuls produce t, 50 matmuls
# contract (g,c) into s, 50 matmuls apply Wa for u_acc, and a ones-matmul /
# GPSIMD partition-reduce handle the cross-partition parts of squash.
# GPSIMD/Act/DVE split the PSUM->SBUF copies and softmax normalization.

import sys

import numpy as np

for _p in ("/opt/trn_rl_repo", "/root/.axon_site/_ro/trn_rl_repo"):
    if _p not in sys.path:
        sys.path.insert(0, _p)

NS, J, D, C_IN, H, WID, RN = 32, 10, 16, 8, 6, 6, 3
S = H * WID              # 36 spatial positions
CA = C_IN + 1            # 9 channels incl. ones
I = NS * S               # 1152 input capsules
NCORES = 8
BLOC = 32                # samples per core
NCHUNK = 9               # i-chunks of 128
NTILE = 5                # (g,c)-row tiles: 2 chunk-slots of 64 rows each
JB = J * BLOC            # 320 free columns, layout (j, b) with b innermost

G_LO = [(128 * k) // S for k in range(NCHUNK)]
G_HI = [(128 * k + 127) // S for k in range(NCHUNK)]
R_LEN = [CA * (G_HI[k] - G_LO[k] + 1) for k in range(NCHUNK)]   # 36/45
TILE_OF = [k // 2 for k in range(NCHUNK)]
SLOT_OF = [64 * (k % 2) for k in range(NCHUNK)]
ROWS_T = [128, 128, 128, 128, 64]

_CACHE = {}


def _build_program(split_waits=True):
    from contextlib import ExitStack

    import concourse.bass as bass
    import concourse.tile as tile
    from concourse import mybir

    f32 = mybir.dt.float32
    f16 = mybir.dt.float16
    Alu = mybir.AluOpType
    Act = mybir.ActivationFunctionType
    AxX = mybir.AxisListType.X
    AxC = mybir.AxisListType.C

    nc = bass.Bass("TRN2", target_bir_lowering=True, debug=False,
                   num_devices=NCORES)

    xe_d = [nc.dram_tensor(f"xe{k}", [R_LEN[k], BLOC * 128], f16,
                           kind="ExternalInput").ap() for k in range(NCHUNK)]
    xb_d = [nc.dram_tensor(f"xb{k}", [128, BLOC * R_LEN[k]], f16,
                           kind="ExternalInput").ap() for k in range(NCHUNK)]
    wc_d = [nc.dram_tensor(f"wc{m}", [ROWS_T[m], J * D], f16,
                           kind="ExternalInput").ap() for m in range(NTILE)]
    wu_d = [nc.dram_tensor(f"wu{m}", [D, J * ROWS_T[m]], f16,
                           kind="ExternalInput").ap() for m in range(NTILE)]
    t1_d = [nc.dram_tensor(f"t1{m}", [ROWS_T[m], BLOC], f16,
                           kind="ExternalInput").ap() for m in range(NTILE)]
    on1_d = nc.dram_tensor("on1", [1, D], f16, kind="ExternalInput").ap()
    v_d = nc.dram_tensor("v", [D, JB], f32, kind="ExternalOutput").ap()

    with tile.TileContext(nc) as tc, ExitStack() as ctx:
        consts = ctx.enter_context(tc.tile_pool(name="consts", bufs=1))
        cpool = ctx.enter_context(tc.tile_pool(name="cpool", bufs=3))
        epool = ctx.enter_context(tc.tile_pool(name="epool", bufs=2))
        small = ctx.enter_context(tc.tile_pool(name="small", bufs=2))
        tspool = ctx.enter_context(tc.tile_pool(name="tspool", bufs=2))
        uspool = ctx.enter_context(tc.tile_pool(name="uspool", bufs=2))
        pg = ctx.enter_context(tc.tile_pool(name="pg", bufs=1, space="PSUM"))
        psq = ctx.enter_context(tc.tile_pool(name="psq", bufs=1,
                                             space="PSUM"))
        pb = ctx.enter_context(tc.tile_pool(name="pb", bufs=2, space="PSUM"))

        dma = nc.sync.dma_start

        on1 = consts.tile([1, D], f16, tag="on1")
        dma(on1[:, :], on1_d[:, :])
        WcS, WuS, T1S = [], [], []
        for m in range(NTILE):
            w = consts.tile([ROWS_T[m], J * D], f16, tag=f"wc{m}")
            dma(w[:, :], wc_d[m][:, :])
            WcS.append(w)
            w = consts.tile([D, J * ROWS_T[m]], f16, tag=f"wu{m}")
            dma(w[:, :], wu_d[m][:, :])
            WuS.append(w)
            w = consts.tile([ROWS_T[m], BLOC], f16, tag=f"t1{m}")
            dma(w[:, :], t1_d[m][:, :])
            T1S.append(w)
        XeS, XbS = [], []
        for k in range(NCHUNK):
            off, rl = SLOT_OF[k], R_LEN[k]
            xe = consts.tile([off + rl, BLOC * 128], f16, tag=f"xe{k}")
            dma(xe[off:off + rl, :], xe_d[k][:, :])
            XeS.append(xe)
            xb = consts.tile([128, BLOC * R_LEN[k]], f16, tag=f"xb{k}")
            dma(xb[:, :], xb_d[k][:, :])
            XbS.append(xb)

        Vacc = consts.tile([D, JB], f16, tag="vacc")

        def copy_eng(n):
            # GPSIMD cannot access PSUM on TRN2; alternate Act/DVE.
            return (nc.scalar, nc.vector)[n % 2]

        def psum_copy(eng, dst, src):
            if eng is nc.scalar:
                eng.copy(dst, src)
            else:
                eng.tensor_copy(dst, src)

        def c_step_and_squash(tS, last, uniform=False):
            """s = t x Wa contracted over (g,c); squash -> v tile.

            tS: NTILE SBUF tiles [rows_m, JB] fp16 ((j,b) cols), or
            [rows_m, BLOC] when uniform (iteration-1 host t,
            j-independent). Returns v tile ([D, JB]; f32 when last).
            """
            s_ps = psq.tile([D, JB], f32, tag="sq", name="s_ps")
            for j in range(J):
                for m in range(NTILE):
                    rhs = (tS[m][:, :] if uniform
                           else tS[m][:, j * BLOC:(j + 1) * BLOC])
                    nc.tensor.matmul(
                        s_ps[:, j * BLOC:(j + 1) * BLOC],
                        WcS[m][:, j * D:(j + 1) * D], rhs,
                        start=(m == 0), stop=(m == NTILE - 1))
            # squash: v = s * sqrt(n2) / (1 + n2), n2 = sum_d s^2
            s2 = small.tile([D, JB], f16, tag="s2")
            nc.scalar.activation(s2[:, :], s_ps[:, :], Act.Square)
            sCp = small.tile([D, JB], f32, tag="sCp")
            nc.scalar.copy(sCp[:, :], s_ps[:, :])
            n2 = small.tile([1, JB], f32, tag="n2")
            with nc.allow_low_precision("fp16 squash intermediates"):
                nc.gpsimd.tensor_reduce(n2[:, :], s2[:, :], AxC, Alu.add)
            a = small.tile([1, JB], f32, tag="sqa")
            nc.scalar.activation(a[:, :], n2[:, :], Act.Sqrt)
            d0 = small.tile([1, JB], f32, tag="d0")
            nc.scalar.add(d0[:, :], n2[:, :], 1.0)
            r = small.tile([1, JB], f32, tag="rcp")
            nc.vector.reciprocal(r[:, :], d0[:, :])
            f = small.tile([1, JB], f16, tag="fac")
            nc.vector.tensor_tensor(f[:, :], a[:, :], r[:, :], Alu.mult)
            f_ps = psq.tile([D, JB], f32, tag="sq", name="f_ps")
            nc.tensor.matmul(f_ps[:, :], on1[:, :], f[:, :],
                             start=True, stop=True)
            vt = small.tile([D, JB], f32 if last else f16, tag="vt")
            nc.vector.tensor_tensor(vt[:, :], sCp[:, :], f_ps[:, :],
                                    Alu.mult)
            return vt

        def u_step():
            """u_acc = Vacc x Wa contracted over d -> NTILE SBUF tiles."""
            u_ps = [pg.tile([ROWS_T[m], JB], f32, tag=f"gc{m}",
                            name=f"ups{m}") for m in range(NTILE)]
            for j in range(J):
                for m in range(NTILE):
                    nc.tensor.matmul(
                        u_ps[m][:, j * BLOC:(j + 1) * BLOC],
                        WuS[m][:, j * ROWS_T[m]:(j + 1) * ROWS_T[m]],
                        Vacc[:, j * BLOC:(j + 1) * BLOC],
                        start=True, stop=True)
            uS = []
            for m in range(NTILE):
                u = uspool.tile([ROWS_T[m], JB], f16, tag=f"us{m}")
                psum_copy(copy_eng(m), u[:, :], u_ps[m][:, :])
                uS.append(u)
            return uS

        def e_heavy(uS):
            """logits B[k] = u_acc x xa, straight into PSUM per chunk."""
            Bs = []
            for k in range(NCHUNK):
                m, off, rl = TILE_OF[k], SLOT_OF[k], R_LEN[k]
                B_ps = pb.tile([128, JB], f32, tag="B", name=f"B{k}")
                u3 = (uS[m][off:off + rl, :]
                      .rearrange("p (j b) -> p j b", j=J))
                for b in range(BLOC):
                    nc.tensor.matmul(
                        B_ps[:, :].rearrange("p (j b) -> p j b", j=J)
                        [:, :, b],
                        XeS[k][off:off + rl, b * 128:(b + 1) * 128],
                        u3[:, :, b], start=True, stop=True)
                Bs.append(B_ps)
            return Bs

        def softmax(k, B_ps):
            """C[k] = softmax_j(B[k]) in [128, (j,b)] fp16."""
            et = epool.tile([128, JB], f16, tag="et", name=f"et{k}")
            nc.scalar.activation(et[:, :], B_ps[:, :], Act.Exp)
            z = small.tile([128, BLOC], f32, tag="z")
            ejb = (et[:, :].rearrange("p (j b) -> p j b", j=J)
                   .transpose([0, 2, 1]))
            nc.vector.tensor_reduce(z[:, :], ejb, AxX, Alu.add)
            zr = small.tile([128, BLOC], f16, tag="zr")
            with nc.allow_low_precision("fp16 softmax normalizer"):
                nc.vector.reciprocal(zr[:, :], z[:, :])
            ct = cpool.tile([128, JB], f16, tag="C", name=f"C{k}")
            zb = zr[:, :].unsqueeze(1).broadcast_to([128, J, BLOC])
            eng = nc.vector if k % 3 == 0 else nc.gpsimd
            eng.tensor_tensor(
                ct[:, :].rearrange("p (j b) -> p j b", j=J),
                et[:, :].rearrange("p (j b) -> p j b", j=J), zb, Alu.mult)
            return ct

        def b_heavy(Cs):
            """t[(g,c)-slots, (j,b)] = C x xa per chunk; PSUM -> SBUF."""
            t_ps = [pg.tile([ROWS_T[m], JB], f32, tag=f"gc{m}",
                            name=f"tps{m}") for m in range(NTILE)]
            for k in range(NCHUNK):
                m, off, rl = TILE_OF[k], SLOT_OF[k], R_LEN[k]
                c3 = Cs[k][:, :].rearrange("p (j b) -> p j b", j=J)
                o3 = (t_ps[m][off:off + rl, :]
                      .rearrange("p (j b) -> p j b", j=J))
                for b in range(BLOC):
                    nc.tensor.matmul(
                        o3[:, :, b],
                        XbS[k][:, b * R_LEN[k]:(b + 1) * R_LEN[k]],
                        c3[:, :, b], start=True, stop=True)
            tS = []
            for m in range(NTILE):
                t = tspool.tile([ROWS_T[m], JB], f16, tag=f"ts{m}")
                psum_copy(copy_eng(m + 2), t[:, :], t_ps[m][:, :])
                tS.append(t)
            return tS

        # ---- iteration 1: softmax(0) is uniform; t1 comes from the host
        vt = c_step_and_squash(T1S, last=False, uniform=True)
        nc.vector.tensor_copy(Vacc[:, :], vt[:, :])

        # ---- iterations 2..3
        for it in range(2, RN + 1):
            uS = u_step()
            Bs = e_heavy(uS)
            Cs = [softmax(k, Bs[k]) for k in range(NCHUNK)]
            tS = b_heavy(Cs)
            vt = c_step_and_squash(tS, last=(it == RN))
            if it == RN:
                dma(v_d[:, :], vt[:, :])
            else:
                nc.vector.tensor_tensor(Vacc[:, :], Vacc[:, :], vt[:, :],
                                        Alu.add)

    if split_waits:
        _split_multi_waits(nc)
    return nc


def _split_multi_waits(nc):
    """Walrus's cayman codegen allows exactly ONE sync wait per TPB
    instruction. Split extras onto engine-local NoOps (semantics
    unchanged); the neuronx-cc path rejects multi-wait instructions."""
    from concourse import mybir

    for bbname, bbwrap in nc.bb_map.items():
        bb = bbwrap.bb
        insts = bb.instructions
        i = 0
        while i < len(insts):
            ins = insts[i]
            si = getattr(ins, "sync_info", None)
            if si is None or len(si.on_wait or []) <= 1:
                i += 1
                continue
            waits = list(si.on_wait)
            engine = ins.engine
            for w in waits[:-1]:
                nop = mybir.InstNoOp(
                    name=nc.get_next_instruction_name(),
                    engine=engine,
                    bass_nofuse=True,
                    sync_info=mybir.SyncInfo(on_wait=[w], on_update=[]),
                )
                insts.insert(i, nop)
                i += 1
            ins.sync_info = mybir.SyncInfo(on_wait=[waits[-1]],
                                           on_update=si.on_update)
            i += 1


def _get_program(split_waits=True):
    key = ("nc", split_waits)
    if key not in _CACHE:
        _CACHE[key] = _build_program(split_waits)
    return _CACHE[key]


def _host_prep(x, W, bias):
    """Stage per-core inputs: block-diagonal x slices as matmul weights,
    Wa arrangements, iteration-1 t (uniform softmax collapsed)."""
    f16 = np.float16
    x = np.ascontiguousarray(x, dtype=np.float32)
    W = np.ascontiguousarray(W, dtype=np.float32)
    bias = np.ascontiguousarray(bias, dtype=np.float32)
    bs = x.shape[0]

    xg = x.reshape(bs, NS, C_IN, S)
    xa = np.concatenate(
        [xg, np.ones((bs, NS, 1, S), dtype=np.float32)], axis=2)

    Wa = np.concatenate(
        [W.reshape(NS, J, D, C_IN), bias.reshape(NS, J, D, 1)],
        axis=3)                                     # [g, j, d, c]

    # static Wa arrangements in the 64-aligned chunk-slot row space
    wc = [np.zeros((ROWS_T[m], J, D), dtype=np.float32)
          for m in range(NTILE)]
    for k in range(NCHUNK):
        m, off = TILE_OF[k], SLOT_OF[k]
        blk = Wa[G_LO[k]:G_HI[k] + 1]               # [span, j, d, c]
        wc[m][off:off + R_LEN[k]] = np.ascontiguousarray(
            blk.transpose(0, 3, 1, 2)).reshape(-1, J, D)
    wc_maps = [np.ascontiguousarray(
        w.reshape(ROWS_T[m], J * D)).astype(f16)
        for m, w in enumerate(wc)]
    wu_maps = [np.ascontiguousarray(
        w.transpose(2, 1, 0).reshape(D, J * ROWS_T[m])).astype(f16)
        for m, w in enumerate(wc)]
    on1 = np.ones((1, D), dtype=f16)

    in_maps = []
    for core in range(NCORES):
        xc = xa[core * BLOC:(core + 1) * BLOC]      # [32, g, c, s]
        mp = {f"wc{m}": wc_maps[m] for m in range(NTILE)}
        mp.update({f"wu{m}": wu_maps[m] for m in range(NTILE)})
        mp["on1"] = on1
        t1 = [np.zeros((ROWS_T[m], BLOC), dtype=np.float32)
              for m in range(NTILE)]
        for k in range(NCHUNK):
            m, soff, rl = TILE_OF[k], SLOT_OF[k], R_LEN[k]
            xe = np.zeros((rl, BLOC, 128), dtype=np.float32)
            xb = np.zeros((128, BLOC, rl), dtype=np.float32)
            for g in range(G_LO[k], G_HI[k] + 1):
                i0 = max(S * g, 128 * k)
                i1 = min(S * g + S, 128 * k + 128)
                s0, s1 = i0 - S * g, i1 - S * g
                r0 = (g - G_LO[k]) * CA
                blk = xc[:, g, :, s0:s1]            # [32, 9, s1-s0]
                xe[r0:r0 + CA, :, i0 - 128 * k:i1 - 128 * k] = \
                    blk.transpose(1, 0, 2)
                xb[i0 - 128 * k:i1 - 128 * k, :, r0:r0 + CA] = \
                    blk.transpose(2, 0, 1)
                t1[m][soff + r0:soff + r0 + CA] = \
                    blk.sum(axis=2).T / J
            mp[f"xe{k}"] = np.ascontiguousarray(
                xe.reshape(rl, BLOC * 128)).astype(f16)
            mp[f"xb{k}"] = np.ascontiguousarray(
                xb.reshape(128, BLOC * rl)).astype(f16)
        for m in range(NTILE):
            mp[f"t1{m}"] = t1[m].astype(f16)
        in_maps.append(mp)
    return in_maps


def kernel(x, W, bias, b0):
    from concourse.bass_utils import run_bass_kernel_spmd

    nc = _get_program()
    in_maps = _host_prep(x, W, bias)
    res = run_bass_kernel_spmd(nc, in_maps, list(range(NCORES)))
    out = np.empty((NCORES * BLOC, J, D), dtype=np.float32)
    for core in range(NCORES):
        v = res.results[core]["v"].reshape(D, J, BLOC)
        out[core * BLOC:(core + 1) * BLOC] = v.transpose(2, 1, 0)
    return out
